# revision 20
# baseline (speedup 1.0000x reference)
"""SAGAN-style self-attention on 8 trn2 cores: data-parallel over batch.

Per core (one batch image): x^T [256,4096] bf16 in, out^T [256,4096] bf16 out.
  QK^T  = [Wf|Wg]^T @ xT        [64, 4096]  one fused matmul per 512-col tile
  V     = x @ Wh                [4096, 256] ([keys, c] layout, 32 tiles of 128)
  per 512-query tile nt, per group g of 4 key blocks:
    T    = KT_blk.T @ QT_tile   4 ADJACENT row-packed matmuls (K=32 strips)
                                into two [128,1024] PSUM tiles tA/tB, so the
                                next T-pack unblocks after one small exp
    e    = exp(tA), exp(tB)     ACT, PSUM->SBUF bf16 (scores ~N(0,0.58^2))
    esum += e(b0)+e(b1)+e(b2)+e(b3)   DVE pairwise tree, running over groups
    O'  += V_blk.T @ e          [256, 512] PSUM accum over 32 key blocks
  Zb   = ones[128,128]^T @ esum [128, 512]  Z broadcast to all partitions in
                                ONE 512-cycle matmul (output rows are free)
  osb  = copy(O')  (post-scale: normalize AFTER the output projection --
                    diag(1/Z) commutes with the channel matmul)
  F    = Wo^T @ osb;  out^T = F * (1/Zb)  (DVE recip + mul);  DMA out bf16.

Tail of tile nt interleaves with T/exp/O' of tile nt+1 (flat software
pipeline, cc-major O' ordering so o_ps drains right at O'-stop).
"""

import sys

if "/opt/trn_rl_repo" not in sys.path:
    sys.path.insert(0, "/opt/trn_rl_repo")

import ml_dtypes
import numpy as np

import concourse.bass as bass
import concourse.mybir as mybir
import concourse.tile as tile
from concourse.bass_utils import run_bass_kernel_spmd

B, H, W, C = 8, 64, 64, 256
KEY = 32
N = H * W          # 4096 tokens
NT = 512           # query tile (free dim per matmul)
NTILES = N // NT   # 8
MB = 128           # key block (contraction chunk)
NMB = N // MB      # 32
GRP = 4            # key blocks per group: one per PE row strip
NGRP = NMB // GRP  # 8 groups per query tile

BF16 = mybir.dt.bfloat16
F32 = mybir.dt.float32
FT = mybir.ActivationFunctionType


def build_nc(has_bias: bool = False, loop_k: int = 0, diag_const_e: bool = False) -> bass.Bass:
    nc = bass.Bass()

    xT = nc.declare_dram_parameter("xT", [2, 128, N], BF16, isOutput=False)
    wfg = nc.declare_dram_parameter("wfg", [2, 128, 2 * KEY], BF16, isOutput=False)
    wh = nc.declare_dram_parameter("wh", [2, 128, C], BF16, isOutput=False)
    wo = nc.declare_dram_parameter("wo", [2, 128, C], BF16, isOutput=False)
    if has_bias:
        bfgp = nc.declare_dram_parameter("bfgp", [2 * KEY, 1], F32, isOutput=False)
        bhp = nc.declare_dram_parameter("bhp", [1, C], BF16, isOutput=False)
        bop = nc.declare_dram_parameter("bop", [2, 128, 1], F32, isOutput=False)
    outT = nc.declare_dram_parameter("outT", [2, 128, N], BF16, isOutput=True)

    with tile.TileContext(nc) as tc:
        with (
            tc.tile_pool(name="const", bufs=1) as const,
            tc.tile_pool(name="xp", bufs=1) as xp,
            tc.tile_pool(name="vp", bufs=1) as vp,
            tc.tile_pool(name="qk", bufs=1) as qk,
            tc.tile_pool(name="ep", bufs=3) as ep,
            tc.tile_pool(name="s1p", bufs=2) as s1p,
            tc.tile_pool(name="s2p", bufs=2) as s2p,
            tc.tile_pool(name="esp", bufs=1) as esp,
            tc.tile_pool(name="osb", bufs=2) as osbp,
            tc.tile_pool(name="zrp", bufs=2) as zrp,
            tc.tile_pool(name="outp", bufs=3) as outp,
            # PSUM budget (8 banks of [128,512]f32):
            #   pt  tags tA/tB        [128,1024] x2 = 4 banks
            #   po  tags o0/o1/f0     [128,512] x1  = 3 banks
            #   pz  tag zb            [128,512] x1  = 1 bank (f1 reuses it)
            # Projections borrow o0/o1 (QK) and tA/zb (V) before attention.
            tc.tile_pool(name="pt", bufs=1, space="PSUM") as pt,
            tc.tile_pool(name="po", bufs=1, space="PSUM") as po,
            tc.tile_pool(name="pz", bufs=1, space="PSUM") as pz,
        ):
            # ---- constants / weights (outside the timing loop) ----
            ones_full = const.tile([128, 128], BF16)  # Zb-matmul lhsT
            nc.vector.memset(ones_full, 1.0)

            wfg_sb = const.tile([128, 2, 2 * KEY], BF16)
            wh_sb = const.tile([128, 2, C], BF16)
            wo_sb = const.tile([128, 2, C], BF16)
            for cc in range(2):
                nc.sync.dma_start(out=wfg_sb[:, cc, :], in_=wfg[cc])
                nc.sync.dma_start(out=wh_sb[:, cc, :], in_=wh[cc])
                nc.scalar.dma_start(out=wo_sb[:, cc, :], in_=wo[cc])
            if has_bias:
                bfg_sb = const.tile([2 * KEY, 1], F32)
                bh_sb = const.tile([1, C], BF16)
                bo_sb = const.tile([2, 128, 1], F32)
                ones_m = const.tile([1, 128], BF16)
                nc.vector.memset(ones_m, 1.0)
                nc.sync.dma_start(out=bfg_sb, in_=bfgp[:])
                nc.sync.dma_start(out=bh_sb, in_=bhp[:])
                nc.sync.dma_start(out=bo_sb, in_=bop[:])
                # bh broadcast to all partitions for the V-proj DVE add
                bh_ps = pz.tile([128, C], F32, tag="zb", name="bhps")
                nc.tensor.matmul(bh_ps, ones_m, bh_sb, start=True, stop=True)
                bh_bc = const.tile([128, C], F32)
                nc.vector.tensor_copy(out=bh_bc, in_=bh_ps)

            def body(it: int):
                sfx = f"i{it}"
                # x^T chunks in 512-col tiles; cc0 on the SP queue, cc1 on
                # the ACT queue so the input lands ~2x faster.
                xts = [
                    [
                        xp.tile([128, NT], BF16, tag=f"xt{cc}_{h}", name=f"xt{cc}_{h}{sfx}")
                        for h in range(NTILES)
                    ]
                    for cc in range(2)
                ]
                for h in range(NTILES):
                    nc.sync.dma_start(out=xts[0][h], in_=xT[0, :, h * NT:(h + 1) * NT])
                    nc.scalar.dma_start(out=xts[1][h], in_=xT[1, :, h * NT:(h + 1) * NT])

                def xs(cc, start, width):
                    h = start // NT
                    assert (start + width - 1) // NT == h
                    return xts[cc][h][:, start - h * NT: start - h * NT + width]

                # ---- fused Q/K projections ----
                # qt_rep [128, N]: Q^T replicated at the four 32-row strip
                # offsets (rhs of the row-packed T matmuls streams from its
                # strip's partitions). kt_stack [128, g, 128]: strip j of
                # group g holds K^T[:, (4g+j)*128:...] -- written DIRECTLY
                # from the projection PSUM (no regroup DMAs).
                # ---- V projection -> 32 tiles [128 keys, 256 c] bf16 ----
                # interleaved into the QK loop (h-major) so both proceed at
                # input-DMA pace instead of serializing QK(0..7) before V.
                v_sb = [None] * NMB
                vtags = ("tA", "zb")

                def v_proj(mb):
                    pool = pt if mb % 2 == 0 else pz
                    ps = pool.tile([128, C], F32, tag=vtags[mb % 2], name=f"vps{mb}{sfx}")
                    for cc in range(2):
                        nc.tensor.matmul(
                            ps, xs(cc, mb * MB, MB), wh_sb[:, cc, :],
                            start=(cc == 0), stop=(cc == 1),
                        )
                    vt = vp.tile([128, C], BF16, tag=f"v{mb}", name=f"v{mb}{sfx}")
                    if has_bias:
                        nc.vector.tensor_add(vt, ps, bh_bc)
                    else:
                        nc.vector.tensor_copy(out=vt, in_=ps)
                    v_sb[mb] = vt

                qt_rep = qk.tile([128, N], BF16, name=f"qt{sfx}")
                kt_stack = qk.tile([128, NGRP, MB], BF16, name=f"kts{sfx}")
                for g in range(NTILES):
                    sl = slice(g * NT, (g + 1) * NT)
                    ps = po.tile([2 * KEY, NT], F32, tag=f"o{g % 2}", name=f"qkps{g}{sfx}")
                    for cc in range(2):
                        nc.tensor.matmul(
                            ps, wfg_sb[:, cc, :], xs(cc, g * NT, NT),
                            start=(cc == 0), stop=(cc == 1),
                        )
                    if has_bias:
                        nc.scalar.activation(
                            out=qt_rep[0:KEY, sl], in_=ps[0:KEY, :],
                            func=FT.Identity, bias=bfg_sb[0:KEY],
                        )
                        for j in range(GRP):
                            nc.scalar.activation(
                                out=kt_stack[KEY * j:KEY * (j + 1), g, :],
                                in_=ps[KEY:2 * KEY, j * MB:(j + 1) * MB],
                                func=FT.Identity, bias=bfg_sb[KEY:2 * KEY],
                            )
                    else:
                        nc.scalar.copy(out=qt_rep[0:KEY, sl], in_=ps[0:KEY, :])
                        for j in range(GRP):
                            nc.scalar.copy(
                                out=kt_stack[KEY * j:KEY * (j + 1), g, :],
                                in_=ps[KEY:2 * KEY, j * MB:(j + 1) * MB],
                            )
                    for l in range(GRP):
                        v_proj(GRP * g + l)
                for i in range(1, 4):
                    nc.sync.dma_start(
                        out=qt_rep[KEY * i:KEY * (i + 1), :], in_=qt_rep[0:KEY, :]
                    )

                # ---- attention: flat pipeline over (query tile, group) ----
                state = {}

                def emit_O(nt, g, eAB):
                    """O' accumulation for group g of tile nt (cc-major)."""
                    if g == 0:
                        state[nt] = [
                            po.tile([128, NT], F32, tag="o0", name=f"o0_{nt}{sfx}"),
                            po.tile([128, NT], F32, tag="o1", name=f"o1_{nt}{sfx}"),
                        ]
                    o_ps = state[nt]
                    for cc in range(2):
                        for j in range(GRP):
                            mb = g * GRP + j
                            nc.tensor.matmul(
                                o_ps[cc],
                                v_sb[mb][:, cc * 128:(cc + 1) * 128],
                                eAB[j // 2][:, (j % 2) * NT:(j % 2 + 1) * NT],
                                start=(mb == 0), stop=(mb == NMB - 1),
                            )

                def tail_a(nt, es):
                    """Zb matmul + o_ps drain copies + reciprocal."""
                    zb_ps = pz.tile([128, NT], F32, tag="zb", name=f"zb{nt}{sfx}")
                    nc.tensor.matmul(zb_ps, ones_full, es, start=True, stop=True)
                    o_ps = state.pop(nt)
                    osb = []
                    for cc in range(2):
                        ot = osbp.tile([128, NT], BF16, tag=f"os{cc}", name=f"os{cc}_{nt}{sfx}")
                        nc.vector.tensor_copy(out=ot, in_=o_ps[cc])
                        osb.append(ot)
                    zr = zrp.tile([128, NT], F32, tag="zr", name=f"zr{nt}{sfx}")
                    nc.vector.reciprocal(out=zr, in_=zb_ps)
                    state[("tail", nt)] = (osb, zr)

                def tail_b(nt):
                    """out projection + normalize + store."""
                    osb, zr = state.pop(("tail", nt))
                    nsl = slice(nt * NT, (nt + 1) * NT)
                    for cp in range(2):
                        csl = slice(cp * 128, (cp + 1) * 128)
                        pool, tag = (po, "f0") if cp == 0 else (pz, "zb")
                        f_ps = pool.tile([128, NT], F32, tag=tag, name=f"f{cp}_{nt}{sfx}")
                        for cc in range(2):
                            nc.tensor.matmul(
                                f_ps, wo_sb[:, cc, csl], osb[cc],
                                start=(cc == 0), stop=(cc == 1),
                            )
                        out_sb = outp.tile([128, NT], BF16, tag="out", name=f"out{cp}_{nt}{sfx}")
                        nc.vector.tensor_mul(out_sb, f_ps, zr)
                        if has_bias:
                            nc.vector.tensor_scalar_add(out_sb, out_sb, bo_sb[cp])
                        nc.sync.dma_start(out=outT[cp, :, nsl], in_=out_sb)

                prev = None
                es_prev = None
                for nt in range(NTILES):
                    nsl = slice(nt * NT, (nt + 1) * NT)
                    for g in range(NGRP):
                        # 4-way row-packed score matmuls: 4 ADJACENT matmuls
                        # into two [128,1024] PSUM tiles -- the tA half
                        # unblocks the next T-pack after one small exp.
                        tA = pt.tile([128, 2 * NT], F32, tag="tA", name=f"tA{nt}_{g}{sfx}")
                        tB = pt.tile([128, 2 * NT], F32, tag="tB", name=f"tB{nt}_{g}{sfx}")
                        for j in range(GRP):
                            dst = tA if j < 2 else tB
                            nc.tensor.matmul(
                                dst[:, (j % 2) * NT:(j % 2 + 1) * NT],
                                kt_stack[KEY * j:KEY * (j + 1), g, :],
                                qt_rep[KEY * j:KEY * (j + 1), nsl],
                                start=True, stop=True,
                                tile_position=(KEY * j, 0),
                            )
                        eA = ep.tile([128, 2 * NT], BF16, tag="eA", name=f"eA{nt}_{g}{sfx}")
                        nc.scalar.activation(out=eA, in_=tA, func=FT.Exp)
                        eB = ep.tile([128, 2 * NT], BF16, tag="eB", name=f"eB{nt}_{g}{sfx}")
                        nc.scalar.activation(out=eB, in_=tB, func=FT.Exp)
                        # running esum over the tile's key blocks (DVE tree)
                        sA = s1p.tile([128, NT], BF16, tag="sA", name=f"sA_{nt}_{g}{sfx}")
                        nc.vector.tensor_add(sA, eA[:, 0:NT], eA[:, NT:2 * NT])
                        sB = s1p.tile([128, NT], BF16, tag="sB", name=f"sB_{nt}_{g}{sfx}")
                        nc.vector.tensor_add(sB, eB[:, 0:NT], eB[:, NT:2 * NT])
                        if g == 0:
                            es = esp.tile([128, NT], BF16, tag="es0", name=f"es{nt}_0{sfx}")
                            nc.vector.tensor_add(es, sA, sB)
                        else:
                            s2 = s2p.tile([128, NT], BF16, tag="s2", name=f"s2_{nt}_{g}{sfx}")
                            nc.vector.tensor_add(s2, sA, sB)
                            es = esp.tile([128, NT], BF16, tag=f"es{g % 2}", name=f"es{nt}_{g}{sfx}")
                            nc.vector.tensor_add(es, s2, es_prev)
                        es_prev = es

                        if prev is not None:
                            pnt, pg, pe = prev
                            emit_O(pnt, pg, pe)
                        if g == 0 and nt > 0:
                            tail_a(nt - 1, state.pop(("es", nt - 1)))
                        if g == 1 and nt > 0:
                            tail_b(nt - 1)
                        prev = (nt, g, (eA, eB))
                    state[("es", nt)] = es_prev
                pnt, pg, pe = prev
                emit_O(pnt, pg, pe)
                tail_a(NTILES - 1, state.pop(("es", NTILES - 1)))
                tail_b(NTILES - 1)

            if loop_k and loop_k > 1:
                with tc.For_i(0, loop_k) as _i:
                    body(0)
            else:
                body(0)

    _split_multiwaits(nc)
    return nc


def _split_multiwaits(nc: bass.Bass) -> None:
    """This container's walrus accepts at most ONE sync-wait per instruction
    (CoreV3GenImpl setupSyncWait). Tile emits multi-wait instructions; split
    the excess waits onto EventSemaphore carriers inserted just before the
    instruction on the same engine — same-engine program order makes this
    semantics-preserving."""
    import json as _json

    data = _json.loads(mybir.module_to_json_bytes(nc.m))
    uid = 0
    for fn in data["functions"]:
        for bb in fn["blocks"]:
            new = []
            for inst in bb["instructions"]:
                si = inst.get("sync_info")
                waits = (si or {}).get("on_wait") or []
                if waits:
                    # Engines execute their queue in order, so a wait on the
                    # instruction's OWN engine semaphore (producers all
                    # earlier in program order) is redundant -- drop it.
                    eng = inst["engine"]
                    kept = [
                        w for w in waits
                        if w.get("ant_name", "").rsplit("_", 1)[0] != eng
                    ]
                    if kept != waits:
                        waits = kept
                        si["on_wait"] = kept
                if len(waits) > 1:
                    for wcmd in waits[:-1]:
                        uid += 1
                        new.append({
                            "debug": inst.get("debug", 0),
                            "engine": inst["engine"],
                            "ins": [], "outs": [],
                            "name": f"syncw-{uid}",
                            "opcode": "EventSemaphore",
                            "sync_info": {"on_update": [], "on_wait": [wcmd]},
                        })
                    si["on_wait"] = [waits[-1]]
                new.append(inst)
            bb["instructions"] = new
    nc.m = mybir.module_from_json_bytes(_json.dumps(data).encode())


_NC = {}


def _get_nc(has_bias: bool = False, loop_k: int = 0, diag_const_e: bool = False):
    key = (has_bias, loop_k, diag_const_e)
    if key not in _NC:
        _NC[key] = build_nc(has_bias, loop_k, diag_const_e)
    return _NC[key]


def _prep_maps(x, Wf, bf, Wg, bg, Wh, bh, Wo, bo):
    bft = ml_dtypes.bfloat16
    has_bias = bool(
        np.any(bf != 0) or np.any(bg != 0) or np.any(bh != 0) or np.any(bo != 0)
    )
    wfg = np.concatenate([Wf, Wg], axis=1)  # [C, 64]
    shared = {
        "wfg": np.ascontiguousarray(wfg.reshape(2, 128, 2 * KEY).astype(bft)),
        "wh": np.ascontiguousarray(Wh.reshape(2, 128, C).astype(bft)),
        "wo": np.ascontiguousarray(Wo.reshape(2, 128, C).astype(bft)),
    }
    if has_bias:
        shared["bfgp"] = np.ascontiguousarray(
            np.concatenate([bf, bg]).reshape(2 * KEY, 1).astype(np.float32)
        )
        shared["bhp"] = np.ascontiguousarray(bh.reshape(1, C).astype(bft))
        shared["bop"] = np.ascontiguousarray(
            bo.reshape(2, 128, 1).astype(np.float32)
        )
    # one vectorized transpose+cast for all batches
    xT_all = np.ascontiguousarray(
        x.reshape(B, N, C).transpose(0, 2, 1).astype(bft)
    ).reshape(B, 2, 128, N)
    in_maps = []
    for b in range(B):
        m = dict(shared)
        m["xT"] = xT_all[b]
        in_maps.append(m)
    return in_maps, has_bias


_CALLABLE = {}


def _get_callable(has_bias: bool):
    """Build (once) and cache the jitted 8-core PJRT callable for the
    straight kernel. run_bass_kernel_spmd rebuilds its jit closure every
    call (full XLA retrace); caching this makes warm calls ~20x faster."""
    if has_bias in _CALLABLE:
        return _CALLABLE[has_bias]

    import jax
    import jax.numpy as jnp
    from jax.sharding import Mesh, PartitionSpec, NamedSharding

    try:
        from jax.experimental.shard_map import shard_map
    except ImportError:
        shard_map = jax.shard_map
    from concourse.bass2jax import (
        _bass_exec_p,
        install_neuronx_cc_hook,
        partition_id_tensor,
    )

    nc = _get_nc(has_bias, 0)
    install_neuronx_cc_hook()
    partition_name = nc.partition_id_tensor.name if nc.partition_id_tensor else None

    in_names, out_names, out_avals, zero_shapes = [], [], [], []
    for alloc in nc.m.functions[0].allocations:
        if not isinstance(alloc, mybir.MemoryLocationSet):
            continue
        name = alloc.memorylocations[0].name
        if alloc.kind == "ExternalInput":
            if name != partition_name:
                in_names.append(name)
        elif alloc.kind == "ExternalOutput":
            out_names.append(name)
            shape = tuple(alloc.tensor_shape)
            dtype = mybir.dt.np(alloc.dtype)
            out_avals.append(jax.core.ShapedArray(shape, dtype))
            zero_shapes.append((shape, dtype))
    n_params = len(in_names)
    n_outs = len(out_avals)
    in_names_full = in_names + out_names
    if partition_name is not None:
        in_names_full.append(partition_name)
    donate = tuple(range(n_params, n_params + n_outs))

    def _body(*args):
        operands = list(args)
        if partition_name is not None:
            operands.append(partition_id_tensor())
        outs = _bass_exec_p.bind(
            *operands,
            out_avals=tuple(out_avals),
            in_names=tuple(in_names_full),
            out_names=tuple(out_names),
            lowering_input_output_aliases=(),
            sim_require_finite=True,
            sim_require_nnan=True,
            nc=nc,
        )
        return tuple(outs)

    devices = jax.devices()[:B]
    mesh = Mesh(np.asarray(devices), ("core",))
    sh = NamedSharding(mesh, PartitionSpec("core"))
    sharded = jax.jit(
        shard_map(
            _body,
            mesh=mesh,
            in_specs=(PartitionSpec("core"),) * (n_params + n_outs),
            out_specs=(PartitionSpec("core"),) * n_outs,
            check_rep=False,
        ),
        donate_argnums=donate,
        keep_unused=True,
    )
    zeros_maker = jax.jit(
        lambda: tuple(
            jnp.zeros((B * s[0], *s[1:]), d) for s, d in zero_shapes
        ),
        out_shardings=tuple(sh for _ in zero_shapes),
    )
    _CALLABLE[has_bias] = (sharded, zeros_maker, in_names, out_names, out_avals, sh)
    return _CALLABLE[has_bias]


class _Res:
    exec_time_ns = None
    profile_json = None

    def __init__(self, results):
        self.results = results


_DEV_CONST = {}


def _run_fast(in_maps, has_bias):
    import jax

    import hashlib

    sharded, zeros_maker, in_names, out_names, out_avals, sh = _get_callable(has_bias)
    hsh = hashlib.md5()
    for name in in_names:
        if name != "xT":
            hsh.update(np.ascontiguousarray(in_maps[0][name]).tobytes())
    wkey = (has_bias, hsh.hexdigest())
    if wkey not in _DEV_CONST:
        _DEV_CONST.clear()
        _DEV_CONST[wkey] = {
            name: jax.device_put(
                np.concatenate([np.asarray(m[name]) for m in in_maps], axis=0), sh
            )
            for name in in_names
            if name != "xT"
        }
    consts = _DEV_CONST[wkey]
    concat_in = [
        consts[name]
        if name != "xT"
        else jax.device_put(
            np.concatenate([np.asarray(m["xT"]) for m in in_maps], axis=0), sh
        )
        for name in in_names
    ]
    outs = sharded(*concat_in, *zeros_maker())
    results = [
        {
            name: np.asarray(outs[i]).reshape(B, *out_avals[i].shape)[c]
            for i, name in enumerate(out_names)
        }
        for c in range(B)
    ]
    return _Res(results)


def run(x, Wf, bf, Wg, bg, Wh, bh, Wo, bo, trace=False, loop_k=0, **kw):
    x = np.asarray(x, dtype=np.float32)
    in_maps, has_bias = _prep_maps(
        x, *(np.asarray(a, dtype=np.float32) for a in (Wf, bf, Wg, bg, Wh, bh, Wo, bo))
    )
    if trace or loop_k or kw:
        nc = _get_nc(has_bias, loop_k)
        res = run_bass_kernel_spmd(nc, in_maps, list(range(B)), trace=trace, **kw)
    else:
        try:
            res = _run_fast(in_maps, has_bias)
        except Exception:
            nc = _get_nc(has_bias, 0)
            res = run_bass_kernel_spmd(nc, in_maps, list(range(B)))
    out = np.empty((B, H, W, C), dtype=np.float32)
    for b in range(B):
        oT = np.asarray(res.results[b]["outT"]).astype(np.float32).reshape(C, N)
        out[b] = oT.T.reshape(H, W, C)
    return out, res


def kernel(x, Wf, bf, Wg, bg, Wh, bh, Wo, bo):
    out, _ = run(x, Wf, bf, Wg, bg, Wh, bh, Wo, bo)
    return out
